# revision 17
# baseline (speedup 1.0000x reference)
"""Trainium2 Bass kernel for the detection loss balancer (nn_Balancer).

Computes: (sum(loss) + 12 * sum(loss * fg_mask)) / (B*H*W)
where fg_mask is, per image, the union of up-to-N axis-aligned boxes
rasterized on the HxW feature grid (box coords / 4, floor/ceil lo/hi).

Strategy (data-parallel over batch, 1 image per NeuronCore):
  - The rectangle-union count is separable: cnt[r,c] = sum_t row_t(r)*col_t(c)
    with row_t = (r>=v1)-(r>=v2), col_t = (c>=u1)-(c>=u2). Expanding the
    product gives 4 signed rank-1 terms per box, evaluated for all 32 boxes
    of an image by a single K=128 TensorEngine matmul per output tile:
       lhsT banks [12A, 12A, -12B, -12B]  (A=(r>=v1), B=(r>=v2); 32 boxes/bank)
       rhs  banks [C, -D, C, -D]          (C=(c>=u1), D=(c>=u2))
    accumulated on top of a K=1 ones x ones "bias" matmul (runs early, while
    the loss DMAs are still in flight), so PSUM holds
       cnt' = 12*cnt + 1,   and the pixel weight is w = min(cnt', 13).
  - Compare banks are built on-device from a GPSIMD iota and a per-core
    bounds vector via one fused tensor_scalar (is_ge, then *sign) per axis.
  - The whole weighted reduction is a handful of fused DVE ops:
       scalar_tensor_tensor: out = (cnt' min 13) * loss,
       accum_out = per-partition row sums  ->  sum(loss*w) directly.
  - Host gathers the 8 cores' [128, n] partial-sum tiles and finishes the
    tiny scalar reduction: result = sum(partials) / (B*H*W).

Layout: 480 rows = 4 row tiles x 120 partitions. loss lives in ONE SBUF tile
[120, 2560] (row tile r at columns [640r, 640r+640)); PSUM is ONE 8-bank tile
[120, 4096] with row tile r at columns [1024r, 1024r+640) so each matmul
col-chunk sits inside a single bank and multi-row-tile STT reads use strided
access patterns.
"""

import numpy as np

try:
    import concourse.bass as bass
except ImportError:  # pragma: no cover - fallback for bare containers
    import sys
    for p in ("/opt/trn_rl_repo", "/root/.axon_site/_ro/trn_rl_repo"):
        if p not in sys.path:
            sys.path.insert(0, p)
    import concourse.bass as bass  # noqa: F401

import concourse.bacc as bacc
import concourse.mybir as mybir
from concourse.tile import TileContext
from concourse.bass_utils import run_bass_kernel_spmd

B, H, W = 8, 480, 640
DOWNSAMPLE = 4
FG_WEIGHT = 13.0
BG_WEIGHT = 1.0
N_CORES = 8

RP = 120                      # rows per tile; 480 = 4*120
N_RT = H // RP                # 4 row tiles
SEG = 1024                    # PSUM col stride per row tile (2 banks)
COL_CHUNKS = [(0, 512), (512, 128)]   # matmul N<=512, bank aligned

# --- schedule knobs (tuned against TimelineSim) ---
STT_GROUPS = ((0,), (1,), (2,), (3,))  # row-tile groups per fused DVE op
BOUNDS_POOL = False  # bounds DMA via gpsimd SWDGE (else SP, issued first)
DMA_ENGINE_PLAN = ("scalar", "sync", "scalar", "sync")  # loss tile issue order
LOSS_BF16_TILES = ()     # row tiles DMA-cast to bf16 via gpsimd (dest bytes halved)
CMPR_ENGINE = "vector"   # engine for the row-compare bank build

_compiled_cache: dict[int, "bass.Bass"] = {}
_TRACE = False      # set True (e.g. from test.py) to capture a HW profile
_last_bkr = None    # last BassKernelResults


def _build_kernel(n_groups: int) -> "bass.Bass":
    """Builds the per-core Bass kernel; n_groups = ceil(boxes_per_image/32)."""
    nc = bacc.Bacc("TRN2", target_bir_lowering=False, debug=False,
                   num_devices=N_CORES)
    dt = mybir.dt
    loss_d = nc.dram_tensor("loss", [H, W], dt.float32, kind="ExternalInput")
    # bounds layout [128, 2G+2] f32; for group g:
    #   col 2g   : row bounds [v1,v1,v2,v2] x32 (banks A,A,B,B)
    #   col 2g+1 : col bounds [u1,u2,u1,u2] x32 (banks C,D,C,D)
    # col 2G = row signs [+12,+12,-12,-12]x32 ; col 2G+1 = [+1,-1,+1,-1]x32
    bounds_d = nc.dram_tensor("bounds", [128, 2 * n_groups + 2], dt.float32,
                              kind="ExternalInput")
    n_acc = len(STT_GROUPS)
    acc_d = nc.dram_tensor("acc", [128, n_acc], dt.float32,
                           kind="ExternalOutput")

    with TileContext(nc) as tc:
        with (
            tc.tile_pool(name="const", bufs=1) as cpool,
            tc.tile_pool(name="lbuf", bufs=1) as lpool,
            tc.tile_pool(name="mbuf", bufs=2 * n_groups) as mpool,
            tc.tile_pool(name="sbuf", bufs=1) as spool,
            tc.tile_pool(name="psum", bufs=1, space="PSUM") as psum,
        ):
            # ones row for the K=1 bias matmuls (DVE is idle this early)
            ones = cpool.tile([1, W], dt.bfloat16, tag="ones")
            nc.vector.memset(ones[:], 1.0)

            bt = cpool.tile([128, 2 * n_groups + 2], dt.float32, tag="bounds")
            (nc.gpsimd if BOUNDS_POOL else nc.sync).dma_start(
                out=bt[:], in_=bounds_d[:])

            # PSUM: one 8-bank tile; bias matmuls fill every pixel with 1.0
            # early, before the box matmuls accumulate 12*cnt on top.
            cnt = psum.tile([RP, N_RT * SEG], dt.float32, tag="cnt")
            for r in range(N_RT):
                for (c0, cw) in COL_CHUNKS:
                    nc.tensor.matmul(
                        cnt[:, r * SEG + c0:r * SEG + c0 + cw],
                        lhsT=ones[:, 0:RP], rhs=ones[:, c0:c0 + cw],
                        start=True, stop=False, skip_group_check=True)

            # loss row tiles; bf16 tiles are DMA-cast via gpsimd SWDGE
            engs = {"sync": nc.sync, "scalar": nc.scalar, "gpsimd": nc.gpsimd}
            plan = [engs[e] for e in DMA_ENGINE_PLAN]
            ltiles = []
            for r in range(N_RT):
                if r in LOSS_BF16_TILES:
                    ltr = lpool.tile([RP, W], dt.bfloat16, tag=f"loss{r}",
                                     name=f"loss_sb{r}")
                    nc.gpsimd.dma_start(
                        out=ltr[:], in_=loss_d[r * RP:(r + 1) * RP, :])
                else:
                    ltr = lpool.tile([RP, W], dt.float32, tag=f"loss{r}",
                                     name=f"loss_sb{r}")
                    plan[r % len(plan)].dma_start(
                        out=ltr[:], in_=loss_d[r * RP:(r + 1) * RP, :])
                ltiles.append(ltr)

            io = cpool.tile([128, W], dt.float32, tag="iota")
            nc.gpsimd.iota(io[:], pattern=[[1, W]], base=0, channel_multiplier=0,
                           allow_small_or_imprecise_dtypes=True)

            acc = cpool.tile([128, n_acc], dt.float32, tag="acc")

            # per-group signed compare banks
            sgr = bt[:, 2 * n_groups:2 * n_groups + 1]
            sgc = bt[:, 2 * n_groups + 1:2 * n_groups + 2]
            cmprs, cmpcs = [], []
            for g in range(n_groups):
                cmpr = mpool.tile([128, H], dt.bfloat16, tag="cmpr",
                                  name=f"cmpr{g}")
                getattr(nc, CMPR_ENGINE).tensor_scalar(
                    out=cmpr[:], in0=io[:, 0:H],
                    scalar1=bt[:, 2 * g:2 * g + 1], scalar2=sgr,
                    op0=mybir.AluOpType.is_ge, op1=mybir.AluOpType.mult)
                cmpc = mpool.tile([128, W], dt.bfloat16, tag="cmpc",
                                  name=f"cmpc{g}")
                nc.vector.tensor_scalar(
                    out=cmpc[:], in0=io[:, 0:W],
                    scalar1=bt[:, 2 * g + 1:2 * g + 2], scalar2=sgc,
                    op0=mybir.AluOpType.is_ge, op1=mybir.AluOpType.mult)
                cmprs.append(cmpr)
                cmpcs.append(cmpc)

            # box matmuls accumulate 12*cnt on top of the bias fill
            for r in range(N_RT):
                r0 = r * RP
                for (c0, cw) in COL_CHUNKS:
                    for g in range(n_groups):
                        nc.tensor.matmul(
                            cnt[:, r * SEG + c0:r * SEG + c0 + cw],
                            lhsT=cmprs[g][:, r0:r0 + RP],
                            rhs=cmpcs[g][:, c0:c0 + cw],
                            start=False, stop=(g == n_groups - 1),
                            skip_group_check=True)

            # fused weighted reductions: accum = sum(loss * min(cnt', 13))
            scr = spool.tile([RP, N_RT * W], dt.float32, tag="scr")
            cnt_s = cnt[:].rearrange("p (s x) -> p s x", x=SEG)
            scr_s = scr[:].rearrange("p (s x) -> p s x", x=W)
            for gi, grp in enumerate(STT_GROUPS):
                s0, ns = grp[0], len(grp)
                assert ns == 1, "mixed-dtype loss tiles need single-tile groups"
                nc.vector.scalar_tensor_tensor(
                    out=scr_s[:, s0:s0 + ns, :],
                    in0=cnt_s[:, s0:s0 + ns, 0:W],
                    scalar=FG_WEIGHT, in1=ltiles[s0][:],
                    op0=mybir.AluOpType.min, op1=mybir.AluOpType.mult,
                    accum_out=acc[0:RP, gi:gi + 1])

            nc.sync.dma_start(out=acc_d[:], in_=acc[:])
    nc.compile()
    return nc


def _box_bounds(boxes: np.ndarray) -> np.ndarray:
    """[n,4] float boxes -> integer bounds (u1,v1,u2,v2) in feature coords."""
    b = boxes.astype(np.float64) / DOWNSAMPLE
    u1 = np.floor(b[:, 0])
    v1 = np.floor(b[:, 1])
    u2 = np.ceil(b[:, 2])
    v2 = np.ceil(b[:, 3])
    return np.stack([u1, v1, u2, v2], axis=1).astype(np.float32)


def kernel(loss: np.ndarray, gt_boxes2d: np.ndarray,
           num_gt_per_img: np.ndarray) -> np.ndarray:
    loss = np.ascontiguousarray(np.asarray(loss, dtype=np.float32))
    boxes = np.asarray(gt_boxes2d, dtype=np.float32).reshape(-1, 4)
    counts = np.asarray(num_gt_per_img).astype(np.int64)
    t_total = boxes.shape[0]

    # replicate jnp.repeat(arange(B), counts, total_repeat_length=T):
    # sequential concat, truncated or padded with the last value.
    reps = np.repeat(np.arange(B), np.clip(counts, 0, None))
    if reps.size >= t_total:
        bids = reps[:t_total]
    elif reps.size == 0:
        bids = np.zeros(t_total, dtype=np.int64)
    else:
        bids = np.concatenate(
            [reps, np.full(t_total - reps.size, reps[-1])])

    per_img = [np.nonzero(bids == b)[0] for b in range(B)]
    max_n = max((len(ix) for ix in per_img), default=0)
    n_groups = max(1, -(-max_n // 32))

    nc = _compiled_cache.get(n_groups)
    if nc is None:
        nc = _build_kernel(n_groups)
        _compiled_cache[n_groups] = nc

    scale = FG_WEIGHT - BG_WEIGHT  # 12: folded into the row-bank signs
    sign_row = np.repeat([scale, scale, -scale, -scale], 32).astype(np.float32)
    sign_col = np.repeat([1.0, -1.0, 1.0, -1.0], 32).astype(np.float32)

    in_maps = []
    for b in range(B):
        bb = _box_bounds(boxes[per_img[b]])  # [n_b, 4] = (u1, v1, u2, v2)
        bounds = np.zeros((128, 2 * n_groups + 2), dtype=np.float32)
        bounds[:, 2 * n_groups] = sign_row
        bounds[:, 2 * n_groups + 1] = sign_col
        for g in range(n_groups):
            chunk = bb[g * 32:(g + 1) * 32]
            k = chunk.shape[0]
            if k == 0:
                continue
            rowv = bounds[:, 2 * g]
            colv = bounds[:, 2 * g + 1]
            rowv[0:k] = chunk[:, 1]        # A: v1
            rowv[32:32 + k] = chunk[:, 1]  # A: v1
            rowv[64:64 + k] = chunk[:, 3]  # B: v2
            rowv[96:96 + k] = chunk[:, 3]  # B: v2
            colv[0:k] = chunk[:, 0]        # C: u1
            colv[32:32 + k] = chunk[:, 2]  # D: u2
            colv[64:64 + k] = chunk[:, 0]  # C: u1
            colv[96:96 + k] = chunk[:, 2]  # D: u2
        in_maps.append({"loss": loss[b], "bounds": bounds})

    global _last_bkr
    _last_bkr = run_bass_kernel_spmd(nc, in_maps, list(range(N_CORES)),
                                     trace=_TRACE)
    results = _last_bkr.results

    total = np.float64(0.0)
    for b in range(B):
        total += results[b]["acc"][0:RP, :].astype(np.float64).sum()
    out = total / (B * H * W)
    return np.asarray(out, dtype=np.float32)
